# revision 19
# baseline (speedup 1.0000x reference)
"""Trainium2 Bass kernel for: out = SCALE * x @ weight.sum(axis=0).

Column-sharded over 8 cores (stripe of 512 cols each). Host precomputes the
scaled column-sum of the weight stripe (weight-only preprocessing, 0.01% of
model FLOPs; all 134 GFLOP of x-dependent work runs on device) and ships it
in the two layouts the engines need:
  wcol [128, 4]  bf16 -- PE lhsT form (wsum chunk c on partitions)
  wsb  [128,512] bf16 -- broadcast form for DVE/Pool scalar_tensor_tensor

Per-core pipeline (batches split across three compute paths):
  PE   (B_PE):  transposed x windows [128, 4, 512]; per 512-batch window 4
                accumulating matmuls lhsT=wcol[:,c], rhs=xw[:,c,:] into a
                one-bank PSUM slice [1, 512].  PSUM: 4 tiles [128, 1024]
                (2 banks), 6 window-slots each (partition {0,32,64} x
                bank-half).  Pool/DVE evict finished tiles with a strided
                [k, 1024] tensor_copy (free-size driven, partition-parallel)
                mid-stream; flat DRAM out-DMAs per tile.
  DVE  (B_DVE): natural x tiles [128, g, 512]; scalar_tensor_tensor with
                in1=wsb, accum_out -> osb_dve column (dot per partition-batch).
  Pool (B_PO):  same stt path on gpsimd between DMA duty.
  PE warmup: ~30 [1,1] matmuls into tile3's unused slot ramp the clock to
  2.4GHz before the first x window lands.
All instruction emission is ordered by a small discrete-event estimator so
each engine's program order matches expected data-arrival order (3 DMA
queues: sync/scalar HWDGE + gpsimd SWDGE).  First PE window / DVE tile are
split/small for fast pipeline fill.  All inputs bf16 (tolerance 2e-2;
observed err ~2e-3).
"""

import numpy as np
import ml_dtypes

from concourse import bacc, bass, tile
import concourse.mybir as mybir
from concourse.bass_utils import run_bass_kernel_spmd

BF16NP = ml_dtypes.bfloat16

N_CORES = 8
BATCH = 16384
IN_SIZE = 4096
CS = IN_SIZE // N_CORES  # 512
W_ROWS = 4096
SCALE = 0.5
P = 128

bf16 = mybir.dt.bfloat16
fp32 = mybir.dt.float32

# --- tunables ---------------------------------------------------------------
B_PE = 11264             # 22 windows of 512; psum tiles of 6/6/6/4 windows
B_DVE = 3584             # 28 stt columns on DVE
B_PO = BATCH - B_PE - B_DVE  # 1536 -> 12 stt columns on Pool
N_WIN = B_PE // 512      # 22
N_DCOL = B_DVE // P      # 28
N_PCOL = B_PO // P       # 12
N_WARM = 30

DVE_TILE_COLS = [2, 3, 4, 4, 4, 4, 4, 3]         # 28
PO_TILE_COLS = [4, 4, 4]                         # 12
assert sum(DVE_TILE_COLS) == N_DCOL and sum(PO_TILE_COLS) == N_PCOL

# psum: big region (banks 0-5) holds windows 0-17; tail tile (banks 6-7)
# holds windows 18-21.  eviction pieces: (tile, slot, n_slots, width,
# after_window); engine chosen dynamically.
N_BIG = 18
EVICT_PIECES = [(0, 0, 3, 3072, 17), (1, 0, 1, 1024, 19), (1, 1, 1, 1024, 21)]

NSB = 0.3855  # dma ns per free byte (v1 cost model)


def build_nc(for_sim: bool = False):
    if for_sim:
        nc = bacc.Bacc(None, target_bir_lowering=False, debug=True, num_devices=1)
    else:
        nc = bacc.Bacc(None, num_devices=N_CORES)

    x_pe_t = nc.declare_dram_parameter("x_pe_t", [CS, B_PE], bf16, isOutput=False)
    x_dve = nc.declare_dram_parameter("x_dve", [B_DVE, CS], bf16, isOutput=False)
    x_po = nc.declare_dram_parameter("x_po", [B_PO, CS], bf16, isOutput=False)
    wcol_e = nc.declare_dram_parameter("wcol_e", [P, 4], bf16, isOutput=False)
    wsb_e = nc.declare_dram_parameter("wsb_e", [P, CS], bf16, isOutput=False)
    out_pe = nc.declare_dram_parameter("out_pe", [B_PE], fp32, isOutput=True)
    out_dve = nc.declare_dram_parameter("out_dve", [P, N_DCOL], fp32, isOutput=True)
    out_po = nc.declare_dram_parameter("out_po", [P, N_PCOL], fp32, isOutput=True)

    with tile.TileContext(nc) as tc:
        with (
            tc.tile_pool(name="xw", bufs=1) as xw_pool,
            tc.tile_pool(name="xd", bufs=1) as xd_pool,
            tc.tile_pool(name="xp", bufs=1) as xp_pool,
            tc.tile_pool(name="aux", bufs=1) as aux,
            tc.tile_pool(name="psum", bufs=1, space="PSUM") as psum,
        ):
            # --- fixed prologue -------------------------------------------
            wcol = aux.tile([P, 4], bf16)
            nc.scalar.dma_start(out=wcol[:], in_=wcol_e[:, :])
            wsb = aux.tile([P, CS], bf16)
            nc.scalar.dma_start(out=wsb[:], in_=wsb_e[:, :])

            ones = aux.tile([P, 1], bf16)
            nc.vector.memset(ones[:], 1.0)

            ps_big = psum.tile([P, 3072], fp32)
            ps_tl = psum.tile([P, 1024], fp32)
            psT = [ps_big, ps_tl]

            # PE warmup: tail-tile slot64 is never used by real windows
            for i in range(N_WARM):
                nc.tensor.matmul(ps_tl[64:65, 520:521], ones[:], ones[:],
                                 start=True, stop=True, skip_group_check=True)

            osb_dve = aux.tile([P, N_DCOL], fp32)
            osb_po = aux.tile([P, N_PCOL], fp32)
            osb_big = aux.tile([P, 3072], fp32)
            osb_tl = aux.tile([P, 1024], fp32)
            osb_pe = [osb_big, osb_tl]

            xw_tiles = [None] * N_WIN
            xd_tiles = [None] * len(DVE_TILE_COLS)
            xp_tiles = [None] * len(PO_TILE_COLS)

            # --- discrete-event ordered emission ---------------------------
            # Engine/queue clocks (estimates; ns).  Latencies approximated.
            clk = {"sync": 100.0, "scalar": 100.0, "pool": 100.0,
                   "pe": 300.0, "dve": 300.0}
            arrival = {}  # ('w'|'d'|'p', i) -> est arrival time

            DMA_LAT = 1800.0

            def pick_queue(extra=()):
                cands = ["sync", "scalar", "pool"]
                return min(cands, key=lambda q: clk[q])

            # window->psum mapping
            w2t = []
            for w in range(N_BIG):
                w2t.append((0, (w // 6) * 32, (w % 6) * 512))
            for w in range(N_BIG, N_WIN):
                k = w - N_BIG
                w2t.append((1, (k // 2) * 32, (k % 2) * 512))

            tile_done_est = [0.0] * 2

            def emit_dma_xw(i, q, lo=0, hi=512):
                if i == 0:
                    # window 0 is two independent half tiles for fast start
                    t = xw_pool.tile([P, 4, hi - lo], bf16,
                                     name=f"xw0{'a' if lo == 0 else 'b'}")
                    qobj = {"sync": nc.sync, "scalar": nc.scalar,
                            "pool": nc.gpsimd}[q]
                    qobj.dma_start(
                        out=t[:],
                        in_=x_pe_t[:, lo:hi]
                        .rearrange("(c p) b -> p c b", p=P))
                    if xw_tiles[0] is None:
                        xw_tiles[0] = [None, None]
                    xw_tiles[0][0 if lo == 0 else 1] = t
                    clk[q] += 4 * (hi - lo) * 2 * NSB
                    arrival[("w", 0)] = max(arrival.get(("w", 0), 0.0),
                                            clk[q] + DMA_LAT)
                    return
                t = xw_tiles[i]
                if t is None:
                    t = xw_pool.tile([P, 4, 512], bf16, name=f"xw{i}")
                    xw_tiles[i] = t
                qobj = {"sync": nc.sync, "scalar": nc.scalar,
                        "pool": nc.gpsimd}[q]
                qobj.dma_start(
                    out=t[:, :, lo:hi],
                    in_=x_pe_t[:, i * 512 + lo:i * 512 + hi]
                    .rearrange("(c p) b -> p c b", p=P))
                cost = 4 * (hi - lo) * 2 * NSB
                clk[q] += cost
                arrival[("w", i)] = max(arrival.get(("w", i), 0.0),
                                        clk[q] + DMA_LAT)

            def emit_dma_xd(i, q):
                g = DVE_TILE_COLS[i]
                off = sum(DVE_TILE_COLS[:i]) * P
                t = xd_pool.tile([P, g, CS], bf16, name=f"xd{i}")
                qobj = {"sync": nc.sync, "scalar": nc.scalar,
                        "pool": nc.gpsimd}[q]
                qobj.dma_start(out=t[:], in_=x_dve[off:off + g * P, :]
                               .rearrange("(g p) f -> p g f", p=P))
                xd_tiles[i] = t
                clk[q] += g * 1024 * NSB
                arrival[("d", i)] = clk[q] + DMA_LAT

            def emit_dma_xp(i, q):
                g = PO_TILE_COLS[i]
                off = sum(PO_TILE_COLS[:i]) * P
                t = xp_pool.tile([P, g, CS], bf16, name=f"xp{i}")
                qobj = {"sync": nc.sync, "scalar": nc.scalar,
                        "pool": nc.gpsimd}[q]
                qobj.dma_start(out=t[:], in_=x_po[off:off + g * P, :]
                               .rearrange("(g p) f -> p g f", p=P))
                xp_tiles[i] = t
                clk[q] += g * 1024 * NSB
                arrival[("p", i)] = clk[q] + DMA_LAT

            def emit_win(w):
                ti, slot, half = w2t[w]
                ps = psT[ti][slot:slot + 1, half:half + 512]
                if w == 0:
                    for h, xt in enumerate(xw_tiles[0]):
                        for c in range(4):
                            nc.tensor.matmul(
                                ps[:, h * 256:(h + 1) * 256],
                                wcol[:, c:c + 1], xt[:, c, :],
                                start=(c == 0), stop=(c == 3),
                                skip_group_check=True)
                else:
                    xt = xw_tiles[w]
                    for c in range(4):
                        nc.tensor.matmul(ps, wcol[:, c:c + 1], xt[:, c, :],
                                         start=(c == 0), stop=(c == 3),
                                         skip_group_check=True)
                st = max(clk["pe"], arrival[("w", w)])
                clk["pe"] = st + 4 * 213.3
                tile_done_est[ti] = clk["pe"]
                win_done_est[w] = clk["pe"]

            def emit_dve(col, i, g):
                nc.vector.scalar_tensor_tensor(
                    out=xd_tiles[i][:, g, :], in0=xd_tiles[i][:, g, :],
                    scalar=1.0, in1=wsb[:],
                    op0=mybir.AluOpType.mult, op1=mybir.AluOpType.mult,
                    accum_out=osb_dve[:, col:col + 1])
                st = max(clk["dve"], arrival[("d", i)])
                clk["dve"] = st + 594.0

            def emit_po(col, i, g):
                nc.gpsimd.scalar_tensor_tensor(
                    out=xp_tiles[i][:, g, :], in0=xp_tiles[i][:, g, :],
                    scalar=1.0, in1=wsb[:],
                    op0=mybir.AluOpType.mult, op1=mybir.AluOpType.mult,
                    accum_out=osb_po[:, col:col + 1])
                st = max(clk["pool"], arrival[("p", i)])
                clk["pool"] = st + 427.0

            win_done_est = [0.0] * N_WIN

            def emit_evict(piece):
                ti, s0, ns, width, _aw = piece
                # pick the engine (pool cheaper per op) by earliest finish
                cp = max(clk["pool"], win_done_est[_aw]) + width * 0.8333
                cd = max(clk["dve"], win_done_est[_aw]) + width * 1.0417 + 130
                eng = "pool" if cp <= cd else "dve"
                eobj = nc.gpsimd if eng == "pool" else nc.vector
                if ns == 1:
                    sl = slice(s0 * 32, s0 * 32 + 1)
                else:
                    sl = slice(s0 * 32, s0 * 32 + (ns - 1) * 32 + 1, 32)
                eobj.tensor_copy(osb_pe[ti][sl, 0:width], psT[ti][sl, 0:width])
                clk[eng] = (cp if eng == "pool" else cd)
                # out-DMAs per partition row: 1-D DRAM writes at 500ns floor
                tbase = 0 if ti == 0 else N_BIG * 512
                for k in range(ns):
                    q = pick_queue()
                    qobj = {"sync": nc.sync, "scalar": nc.scalar,
                            "pool": nc.gpsimd}[q]
                    row = (s0 + k) * 32
                    base = tbase + (s0 + k) * width
                    qobj.dma_start(
                        out=out_pe[base: base + width],
                        in_=osb_pe[ti][row:row + 1, 0:width])
                    clk[q] += 500.0

            # ---- build worklists -----------------------------------------
            # DMA pieces in priority order (fast-start first)
            dma_q = []
            dma_q.append(("w0a", lambda q: emit_dma_xw(0, q, 0, 256)))
            dma_q.append(("w0b", lambda q: emit_dma_xw(0, q, 256, 512)))
            dma_q.append(("d0", lambda q: emit_dma_xd(0, q)))
            wi, di, pi_, step = 1, 1, 0, 0
            pat = ["P", "D", "P", "P", "D", "P", "P", "D", "P", "D"]
            while wi < N_WIN or di < len(DVE_TILE_COLS) or pi_ < len(PO_TILE_COLS):
                kind = pat[step % len(pat)]
                step += 1
                if step in (8, 16, 24) and pi_ < len(PO_TILE_COLS):
                    dma_q.append((f"p{pi_}",
                                  lambda q, j=pi_: emit_dma_xp(j, q)))
                    pi_ += 1
                if kind == "P" and wi < N_WIN:
                    dma_q.append((f"w{wi}",
                                  lambda q, j=wi: emit_dma_xw(j, q)))
                    wi += 1
                elif di < len(DVE_TILE_COLS):
                    dma_q.append((f"d{di}",
                                  lambda q, j=di: emit_dma_xd(j, q)))
                    di += 1
                elif wi < N_WIN:
                    dma_q.append((f"w{wi}",
                                  lambda q, j=wi: emit_dma_xw(j, q)))
                    wi += 1
                elif pi_ < len(PO_TILE_COLS):
                    dma_q.append((f"p{pi_}",
                                  lambda q, j=pi_: emit_dma_xp(j, q)))
                    pi_ += 1

            # compute worklists
            win_q = list(range(N_WIN))
            evict_work = list(EVICT_PIECES)
            dve_q = [(col, i, g) for col, (i, g) in enumerate(
                (i, g) for i in range(len(DVE_TILE_COLS))
                for g in range(DVE_TILE_COLS[i]))]
            po_q = [(col, i, g) for col, (i, g) in enumerate(
                (i, g) for i in range(len(PO_TILE_COLS))
                for g in range(PO_TILE_COLS[i]))]


            # ---- event loop: emit items in estimated start order ---------
            emitted_w = set()
            emitted_d = set()
            emitted_p = set()

            def est_next():
                cands = []
                if dma_q:
                    q = pick_queue()
                    cands.append((clk[q], "dma"))
                if win_q:
                    w = win_q[0]
                    if ("w", w) in arrival:
                        cands.append((max(clk["pe"], arrival[("w", w)]),
                                      "win"))
                if dve_q:
                    col, i, g = dve_q[0]
                    if ("d", i) in arrival:
                        cands.append((max(clk["dve"], arrival[("d", i)]),
                                      "dve"))
                if po_q:
                    col, i, g = po_q[0]
                    if ("p", i) in arrival:
                        cands.append((max(clk["pool"], arrival[("p", i)]),
                                      "po"))
                if evict_work:
                    piece = evict_work[0]
                    if piece[4] not in win_pending:
                        est = max(min(clk["pool"], clk["dve"]),
                                  win_done_est[piece[4]])
                        cands.append((est, "evict"))
                return min(cands, key=lambda c: c[0])[1] if cands else None

            win_pending = set(range(N_WIN))
            n_dma_pieces = len(dma_q)
            while dma_q or win_q or dve_q or po_q or evict_work:
                kind = est_next()
                if kind == "dma":
                    name, fn = dma_q.pop(0)
                    fn(pick_queue())
                elif kind == "win":
                    w = win_q.pop(0)
                    emit_win(w)
                    win_pending.discard(w)
                elif kind == "dve":
                    col, i, g = dve_q.pop(0)
                    emit_dve(col, i, g)
                elif kind == "po":
                    col, i, g = po_q.pop(0)
                    emit_po(col, i, g)
                elif kind == "evict":
                    emit_evict(evict_work.pop(0))
                else:
                    raise AssertionError("stalled emission")

            # final small outs (split: bulk early-ish, small final piece)
            nc.sync.dma_start(out=out_dve[:, 0:24], in_=osb_dve[:, 0:24])
            nc.sync.dma_start(out=out_dve[:, 24:], in_=osb_dve[:, 24:])
            nc.scalar.dma_start(out=out_po[:, 0:8], in_=osb_po[:, 0:8])
            nc.gpsimd.dma_start(out=out_po[:, 8:], in_=osb_po[:, 8:])

    return nc


_NC_CACHE: dict = {}


def _get_nc():
    if "nc" not in _NC_CACHE:
        nc = build_nc()
        nc.finalize()
        _NC_CACHE["nc"] = nc
    return _NC_CACHE["nc"]


def make_in_maps(x: np.ndarray, weight: np.ndarray):
    wsum = weight.astype(np.float64).sum(axis=0) * SCALE  # [4096]
    maps = []
    for c in range(N_CORES):
        sl = slice(c * CS, (c + 1) * CS)
        xs = x[:, sl].astype(BF16NP)
        ws = wsum[sl].astype(BF16NP)  # [512]
        wcol = np.ascontiguousarray(ws.reshape(4, P).T)        # [128, 4]
        wsb = np.ascontiguousarray(np.broadcast_to(ws, (P, CS)))
        maps.append({
            "x_pe_t": np.ascontiguousarray(xs[:B_PE].T),
            "x_dve": np.ascontiguousarray(xs[B_PE:B_PE + B_DVE]),
            "x_po": np.ascontiguousarray(xs[B_PE + B_DVE:]),
            "wcol_e": wcol,
            "wsb_e": wsb,
        })
    return maps


def assemble(results) -> np.ndarray:
    out = np.zeros(BATCH, dtype=np.float64)
    for cid in range(N_CORES):
        r = results[cid]
        out[:B_PE] += np.asarray(r["out_pe"], dtype=np.float64)
        out[B_PE:B_PE + B_DVE] += (
            np.asarray(r["out_dve"], dtype=np.float64).T.reshape(-1))
        out[B_PE + B_DVE:] += (
            np.asarray(r["out_po"], dtype=np.float64).T.reshape(-1))
    return out.astype(np.float32)


def kernel(x: np.ndarray, weight: np.ndarray) -> np.ndarray:
    x = np.asarray(x, dtype=np.float32)
    weight = np.asarray(weight, dtype=np.float32)
    assert x.shape == (BATCH, IN_SIZE) and weight.shape == (W_ROWS, IN_SIZE)
    nc = _get_nc()
    res = run_bass_kernel_spmd(nc, make_in_maps(x, weight),
                               list(range(N_CORES))).results
    return assemble(res)
